# revision 1
# baseline (speedup 1.0000x reference)
"""DA-RNN (dual-stage attention RNN) forward, data-parallel over batch on 8 NeuronCores.

Strategy (per sharding hint): batch B=512 is split 64-per-core across the 8
cores; all weights are replicated. Recurrent state (h, c, context) and both
attention blocks are independent per batch element, so no cross-core
communication is needed; outputs are concatenated on the host.

Algebraic note: the encoder input-attention score `h@Wh + c@Wc + b` is a
per-row constant inside a softmax over features, so it cancels exactly:
attn = softmax(series_score). The encoder attention is therefore constant
over time and the input-side LSTM projection is hoisted out of the scan as
one large matmul.
"""

import numpy as np
import jax
import jax.numpy as jnp
from jax import lax

B, TM1, NTS, NIN, H, D = 512, 64, 64, 63, 128, 128
E = H
NCORES = 8
BL = B // NCORES  # 64 per core


def _lstm_step(xt, h, c, Wih, Whh, b):
    g = xt @ Wih.T + h @ Whh.T + b
    i, f, gg, o = jnp.split(g, 4, axis=-1)
    c = jax.nn.sigmoid(f) * c + jax.nn.sigmoid(i) * jnp.tanh(gg)
    h = jax.nn.sigmoid(o) * jnp.tanh(c)
    return h, c


def _forward_local(x, enc_attn_W, enc_attn_b, enc_Wih, enc_Whh, enc_b,
                   dec_W1, dec_b1, dec_W2, dec_b2, dec_Wih, dec_Whh, dec_b,
                   fc_W, fc_b, fcf_W, fcf_b):
    """x: (BL, TM1, NTS) local batch shard; returns (BL, 1)."""
    xin = x[:, :, 1:]            # (BL, T-1, NIN)
    y_hist = x[:, :, :1]         # (BL, T-1, 1)
    z0 = jnp.zeros((xin.shape[0], H), x.dtype)

    # ---- Encoder ----
    Wt = enc_attn_W[0, 2 * H:]
    series_score = jnp.einsum('btn,t->bn', xin, Wt) + enc_attn_b[0]
    attn = jax.nn.softmax(series_score, axis=1)          # (BL, NIN), const in t
    wi = attn[:, None, :] * xin                          # (BL, T-1, NIN)
    # hoist the input projection out of the recurrence
    xp = jnp.einsum('btn,gn->btg', wi, enc_Wih) + enc_b  # (BL, T-1, 4H)

    def enc_step(carry, xpt):
        h, c = carry
        g = xpt + h @ enc_Whh.T
        i, f, gg, o = jnp.split(g, 4, axis=-1)
        c = jax.nn.sigmoid(f) * c + jax.nn.sigmoid(i) * jnp.tanh(gg)
        h = jax.nn.sigmoid(o) * jnp.tanh(c)
        return (h, c), h

    _, enc_hs = lax.scan(enc_step, (z0, z0), xp.transpose(1, 0, 2))
    input_encoded = enc_hs.transpose(1, 0, 2)            # (BL, T-1, H)

    # ---- Decoder ----
    W1h, W1c, W1e = dec_W1[:, :D], dec_W1[:, D:2 * D], dec_W1[:, 2 * D:]
    enc_proj = jnp.einsum('bte,fe->btf', input_encoded, W1e) + dec_b1

    def dec_step(carry, yt):
        h, c, _ = carry
        z = jnp.tanh(enc_proj + (h @ W1h.T + c @ W1c.T)[:, None, :])
        score = jnp.einsum('bte,e->bt', z, dec_W2[0]) + dec_b2[0]
        attn_t = jax.nn.softmax(score, axis=1)
        context = jnp.einsum('bt,bte->be', attn_t, input_encoded)
        y_tilde = jnp.concatenate([context, yt], axis=1) @ fc_W.T + fc_b
        h, c = _lstm_step(y_tilde, h, c, dec_Wih, dec_Whh, dec_b)
        return (h, c, context), None

    (h, c, context), _ = lax.scan(
        dec_step, (z0, z0, jnp.zeros((xin.shape[0], E), x.dtype)),
        y_hist.transpose(1, 0, 2))

    return jnp.concatenate([h, context], axis=1) @ fcf_W.T + fcf_b


_pforward = jax.pmap(_forward_local, axis_name='i',
                     in_axes=(0,) + (None,) * 16)


def kernel(x, enc_attn_W, enc_attn_b, enc_Wih, enc_Whh, enc_bih, enc_bhh,
           dec_W1, dec_b1, dec_W2, dec_b2, dec_Wih, dec_Whh, dec_bih, dec_bhh,
           fc_W, fc_b, fcf_W, fcf_b):
    x = np.asarray(x, dtype=np.float32)
    xs = x.reshape(NCORES, BL, TM1, NTS)
    enc_b = np.asarray(enc_bih) + np.asarray(enc_bhh)
    dec_b = np.asarray(dec_bih) + np.asarray(dec_bhh)
    out = _pforward(jnp.asarray(xs),
                    jnp.asarray(enc_attn_W), jnp.asarray(enc_attn_b),
                    jnp.asarray(enc_Wih), jnp.asarray(enc_Whh),
                    jnp.asarray(enc_b),
                    jnp.asarray(dec_W1), jnp.asarray(dec_b1),
                    jnp.asarray(dec_W2), jnp.asarray(dec_b2),
                    jnp.asarray(dec_Wih), jnp.asarray(dec_Whh),
                    jnp.asarray(dec_b),
                    jnp.asarray(fc_W), jnp.asarray(fc_b),
                    jnp.asarray(fcf_W), jnp.asarray(fcf_b))
    return np.asarray(out).reshape(B, 1)

